# revision 14
# baseline (speedup 1.0000x reference)
"""EMA (exponential moving average) linear recurrence on 8 trn2 NeuronCores.

y[0] = x[0]; y[t] = s*x[t] + (1-s)*y[t-1],  s = 0.3, x: (64, 4096, 256) fp32.

Algorithm: with a = 1-s = 0.7, a^128 ~ 1.6e-20, so history beyond 256 steps is
far below fp32 resolution. Chunk T into blocks of L=128 and write the scan as a
blocked FIR evaluated on the TensorEngine:

    y_c = M @ x_c + P @ x_{c-1}        (chunk 0: y_0 = M0 @ x_0)

with constant 128x128 matrices
    M[i,j]  = s * a^(i-j)   (j <= i),   M0 = M with column 0 scaled to a^i
    P[i,j]  = s * a^(i+128-j)           (dropped terms <= s*a^256 ~ 1e-40)

Sharding: batch B=64 split across the 8 cores (8 rows each); the recurrence is
along T only, so no cross-core communication is needed.

Precision: matmuls run in plain fp16 (1 cyc/row on the PE vs 4 for fp32, and
fp16 weights get fast-weight-load): x is cast to fp16 on ACT and multiplied by
fp16-rounded weights, giving ~5e-4 relative error -- far inside the 2e-2 gate.
The previous 6-pass hi/lo split variant (3e-7 error) made the TensorEngine
(5.6us/chunk) gate the store drain after loads finish; at 2 passes/chunk
(~1.9us) the kernel is purely DMA-bound (~64 MiB/core at ~358 GB/s).
"""
import numpy as np

import concourse.bacc as bacc
import concourse.mybir as mybir
from concourse import tile
from concourse.bass_utils import run_bass_kernel_spmd

S = 0.3
A = 1.0 - S
B, T, D = 64, 4096, 256
NCORES = 8
BC = B // NCORES          # 8 batch rows per core
L = 128                   # chunk length along T == matmul contraction dim
NCH = T // L              # 32 chunks
CB = BC * D               # 2048 free elements per chunk
NSL = CB // 512           # 4 matmul slices (one PSUM bank each)

f32 = mybir.dt.float32
f16 = mybir.dt.float16

_nc_cache = []


def _weights():
    i = np.arange(L, dtype=np.float64)[:, None]
    j = np.arange(L, dtype=np.float64)[None, :]
    M = np.where(j <= i, S * A ** (i - j), 0.0)
    M0 = M.copy()
    M0[:, 0] = A ** i[:, 0]
    P = S * A ** (i + L - j)

    def half(w):
        # lhsT layout [K, M_out] = W.T, rounded to fp16
        return np.ascontiguousarray(w.T.astype(np.float16))

    return half(M0), half(M), half(P)


def _build():
    nc = bacc.Bacc("TRN2", target_bir_lowering=False, debug=False)
    x = nc.dram_tensor("x", [BC, T, D], f32, kind="ExternalInput").ap()
    wnames = ("wm0", "wm", "wp")
    # all three weight matrices in one tensor -> one DMA at kernel start
    wall = nc.dram_tensor("wall", [L, 3 * L], f16, kind="ExternalInput").ap()
    y = nc.dram_tensor("y", [BC, T, D], f32, kind="ExternalOutput").ap()

    with tile.TileContext(nc) as tc, \
         tc.tile_pool(name="w", bufs=1) as wpool, \
         tc.tile_pool(name="xs", bufs=8) as xpool, \
         tc.tile_pool(name="xh", bufs=6) as xhpool, \
         tc.tile_pool(name="ys", bufs=9) as ypool, \
         tc.tile_pool(name="ps", bufs=2, space="PSUM") as pspool:
        wall_t = wpool.tile([L, 3 * L], f16)
        # first in the sync-ring queue: small, lands before chunk 0
        nc.sync.dma_start(wall_t[:], wall[:])
        wt = {n: wall_t[:, k * L:(k + 1) * L] for k, n in enumerate(wnames)}

        def load_and_cast(c):
            xt = xpool.tile([L, CB], f32, name=f"xt{c}", tag="xt")
            # DRAM view [p(t), b, d]: 3D AP, 1 KiB contiguous runs
            src = x[:, c * L:(c + 1) * L, :].rearrange("b p d -> p b d")
            xh = xhpool.tile([L, CB], f16, name=f"xh{c}", tag="xh")
            if c == 0:
                # chunk 0 gates PE start: pipeline it at 512-element slices
                for n in range(NSL):
                    sl = slice(n * 512, (n + 1) * 512)
                    nc.sync.dma_start(
                        xt[:, sl].rearrange("p (b d) -> p b d", b=2, d=D),
                        src[:, 2 * n:2 * n + 2, :],
                    )
                    nc.vector.tensor_copy(xh[:, sl], xt[:, sl])
            else:
                nc.sync.dma_start(xt[:].rearrange("p (b d) -> p b d", b=BC), src)
                nc.vector.tensor_copy(xh[:], xt[:])     # DVE: fp16 cast
            return xh

        casts = {0: load_and_cast(0)}
        prev_xh = None
        for c in range(NCH):
            # emit next chunk's load+cast BEFORE this chunk's matmuls so the
            # cast sits ahead of the evac work in the engine FIFOs (no
            # PE -> evac -> cast -> PE serialization).
            if c + 1 < NCH:
                casts[c + 1] = load_and_cast(c + 1)
            xh = casts.pop(c)

            ps = pspool.tile([L, CB], f32)
            mh = wt["wm0"] if c == 0 else wt["wm"]
            for n in range(NSL):
                nc.tensor.matmul(
                    ps[:, n * 512:(n + 1) * 512], mh,
                    xh[:, n * 512:(n + 1) * 512],
                    start=True, stop=(c == 0),
                )
            if c > 0:
                for n in range(NSL):
                    nc.tensor.matmul(
                        ps[:, n * 512:(n + 1) * 512], wt["wp"],
                        prev_xh[:, n * 512:(n + 1) * 512],
                        start=False, stop=True,
                    )

            # per-slice evac + store on two engine rings so the PSUM->SBUF
            # copy and the store dispatch never serialize a full chunk
            yt = ypool.tile([L, CB], f32)
            dst = y[:, c * L:(c + 1) * L, :].rearrange("b p d -> p b d")
            for n in range(NSL):
                sl = slice(n * 512, (n + 1) * 512)
                if n < 2:
                    nc.scalar.copy(yt[:, sl], ps[:, sl])
                else:
                    nc.vector.tensor_copy(yt[:, sl], ps[:, sl])
                (nc.scalar if n < 2 else nc.gpsimd).dma_start(
                    dst[:, 2 * n:2 * n + 2, :],
                    yt[:, sl].rearrange("p (b d) -> p b d", b=2, d=D),
                )
            prev_xh = xh
    nc.compile()
    return nc


def get_nc():
    if not _nc_cache:
        _nc_cache.append(_build())
    return _nc_cache[0]


def make_in_maps(x: np.ndarray):
    x = np.ascontiguousarray(np.asarray(x), dtype=np.float32)
    assert x.shape == (B, T, D)
    wm0, wm, wp = _weights()
    wall = np.ascontiguousarray(np.concatenate([wm0, wm, wp], axis=1))
    return [{"x": x[i * BC:(i + 1) * BC], "wall": wall} for i in range(NCORES)]


def kernel(x: np.ndarray) -> np.ndarray:
    res = run_bass_kernel_spmd(
        get_nc(), make_in_maps(x), list(range(NCORES))
    ).results
    return np.concatenate([res[i]["y"] for i in range(NCORES)], axis=0)



# revision 16
# speedup vs baseline: 1.2816x; 1.2816x over previous
"""EMA (exponential moving average) linear recurrence on 8 trn2 NeuronCores.

y[0] = x[0]; y[t] = s*x[t] + (1-s)*y[t-1],  s = 0.3, x: (64, 4096, 256) fp32.

Algorithm: with a = 1-s = 0.7, a^128 ~ 1.6e-20, so history beyond 256 steps is
far below fp32 resolution. Chunk T into blocks of L=128 and write the scan as a
blocked FIR evaluated on the TensorEngine:

    y_c = M @ x_c + P @ x_{c-1}        (chunk 0: y_0 = M0 @ x_0)

with constant 128x128 matrices
    M[i,j]  = s * a^(i-j)   (j <= i),   M0 = M with column 0 scaled to a^i
    P[i,j]  = s * a^(i+128-j)           (dropped terms <= s*a^256 ~ 1e-40)

Sharding: batch B=64 split across the 8 cores (8 rows each); the recurrence is
along T only, so no cross-core communication is needed.

Precision: matmuls run in plain fp16 (1 cyc/row on the PE vs 4 for fp32, and
fp16 weights get fast-weight-load): x is cast to fp16 on ACT and multiplied by
fp16-rounded weights, giving ~5e-4 relative error -- far inside the 2e-2 gate.
The previous 6-pass hi/lo split variant (3e-7 error) made the TensorEngine
(5.6us/chunk) gate the store drain after loads finish; at 2 passes/chunk
(~1.9us) the kernel is purely DMA-bound (~64 MiB/core at ~358 GB/s).
"""
import numpy as np

import concourse.bacc as bacc
import concourse.mybir as mybir
from concourse import tile
from concourse.bass_utils import run_bass_kernel_spmd

S = 0.3
A = 1.0 - S
B, T, D = 64, 4096, 256
NCORES = 8
BC = B // NCORES          # 8 batch rows per core
L = 128                   # chunk length along T == matmul contraction dim
NCH = T // L              # 32 chunks
CB = BC * D               # 2048 free elements per chunk
NSL = CB // 512           # 4 matmul slices (one PSUM bank each)

f32 = mybir.dt.float32
f16 = mybir.dt.float16

_nc_cache = []


def _weights():
    i = np.arange(L, dtype=np.float64)[:, None]
    j = np.arange(L, dtype=np.float64)[None, :]
    M = np.where(j <= i, S * A ** (i - j), 0.0)
    M0 = M.copy()
    M0[:, 0] = A ** i[:, 0]
    P = S * A ** (i + L - j)

    def half(w):
        # lhsT layout [K, M_out] = W.T, rounded to fp16
        return np.ascontiguousarray(w.T.astype(np.float16))

    return half(M0), half(M), half(P)


def _build():
    nc = bacc.Bacc("TRN2", target_bir_lowering=False, debug=False)
    x = nc.dram_tensor("x", [BC, T, D], f32, kind="ExternalInput").ap()
    wnames = ("wm0", "wm", "wp")
    # all three weight matrices in one tensor -> one DMA at kernel start
    wall = nc.dram_tensor("wall", [L, 3 * L], f16, kind="ExternalInput").ap()
    y = nc.dram_tensor("y", [BC, T, D], f32, kind="ExternalOutput").ap()

    with tile.TileContext(nc) as tc, \
         tc.tile_pool(name="w", bufs=1) as wpool, \
         tc.tile_pool(name="xs", bufs=8) as xpool, \
         tc.tile_pool(name="xh", bufs=6) as xhpool, \
         tc.tile_pool(name="ys", bufs=9) as ypool, \
         tc.tile_pool(name="ps", bufs=2, space="PSUM") as pspool:
        wall_t = wpool.tile([L, 3 * L], f16)
        # first in the sync-ring queue: small, lands before chunk 0
        nc.sync.dma_start(wall_t[:], wall[:])
        wt = {n: wall_t[:, k * L:(k + 1) * L] for k, n in enumerate(wnames)}

        def load_and_cast(c):
            xt = xpool.tile([L, CB], f32, name=f"xt{c}", tag="xt")
            # DRAM view [p(t), b, d]: 3D AP, 1 KiB contiguous runs
            src = x[:, c * L:(c + 1) * L, :].rearrange("b p d -> p b d")
            xh = xhpool.tile([L, CB], f16, name=f"xh{c}", tag="xh")
            if c == 0:
                # chunk 0 gates PE start: pipeline it at 512-element slices
                for n in range(NSL):
                    sl = slice(n * 512, (n + 1) * 512)
                    nc.sync.dma_start(
                        xt[:, sl].rearrange("p (b d) -> p b d", b=2, d=D),
                        src[:, 2 * n:2 * n + 2, :],
                    )
                    nc.vector.tensor_copy(xh[:, sl], xt[:, sl])
            else:
                nc.sync.dma_start(xt[:].rearrange("p (b d) -> p b d", b=BC), src)
                nc.vector.tensor_copy(xh[:], xt[:])     # DVE: fp16 cast
            return xh

        casts = {0: load_and_cast(0)}
        prev_xh = None
        for c in range(NCH):
            # emit next chunk's load+cast BEFORE this chunk's matmuls so the
            # cast sits ahead of the evac work in the engine FIFOs (no
            # PE -> evac -> cast -> PE serialization).
            if c + 1 < NCH:
                casts[c + 1] = load_and_cast(c + 1)
            xh = casts.pop(c)

            ps = pspool.tile([L, CB], f32)
            mh = wt["wm0"] if c == 0 else wt["wm"]
            for n in range(NSL):
                nc.tensor.matmul(
                    ps[:, n * 512:(n + 1) * 512], mh,
                    xh[:, n * 512:(n + 1) * 512],
                    start=True, stop=(c == 0),
                )
            if c > 0:
                # P's significant entries live in j >= 64 (dropped terms
                # are <= s*a^65 ~ 2.5e-10): K=64 matmul on partitions 64:128.
                # Also keeps total PE row-activity under the HAM throttle
                # trip point (K=128 here trips 50% DMA duty-cycling).
                for n in range(NSL):
                    nc.tensor.matmul(
                        ps[:, n * 512:(n + 1) * 512], wt["wp"][64:128, :],
                        prev_xh[64:128, n * 512:(n + 1) * 512],
                        start=False, stop=True,
                    )

            # per-slice evac + store on two engine rings so the PSUM->SBUF
            # copy and the store dispatch never serialize a full chunk
            yt = ypool.tile([L, CB], f32)
            dst = y[:, c * L:(c + 1) * L, :].rearrange("b p d -> p b d")
            for n in range(NSL):
                sl = slice(n * 512, (n + 1) * 512)
                if n < 2:
                    nc.scalar.copy(yt[:, sl], ps[:, sl])
                else:
                    nc.vector.tensor_copy(yt[:, sl], ps[:, sl])
                (nc.scalar if n < 2 else nc.gpsimd).dma_start(
                    dst[:, 2 * n:2 * n + 2, :],
                    yt[:, sl].rearrange("p (b d) -> p b d", b=2, d=D),
                )
            prev_xh = xh
    nc.compile()
    return nc


def get_nc():
    if not _nc_cache:
        _nc_cache.append(_build())
    return _nc_cache[0]


def make_in_maps(x: np.ndarray):
    x = np.ascontiguousarray(np.asarray(x), dtype=np.float32)
    assert x.shape == (B, T, D)
    wm0, wm, wp = _weights()
    wall = np.ascontiguousarray(np.concatenate([wm0, wm, wp], axis=1))
    return [{"x": x[i * BC:(i + 1) * BC], "wall": wall} for i in range(NCORES)]


def kernel(x: np.ndarray) -> np.ndarray:
    res = run_bass_kernel_spmd(
        get_nc(), make_in_maps(x), list(range(NCORES))
    ).results
    return np.concatenate([res[i]["y"] for i in range(NCORES)], axis=0)

